# revision 1
# baseline (speedup 1.0000x reference)
"""Distributed multi-head attention for 8 TRN2 NeuronCores.

Problem: x[2,2048,1024] -> QKV proj (w_qkv[3072,1024]) -> 16-head SDPA ->
out proj (w_proj[1024,1024] + b_proj) -> [2,2048,1024].

Sharding: 2 heads per core (head-parallel over all 8 cores; both batches on
every core). Per core:
  Phase A: qT/kT [128(=2 heads x 64d), 4096] and V-natural [4096, 128] from
           x @ w_qkv_shard.T (fp32r matmuls; V via PE transpose of vT).
  Phase B: transposed-score attention per (batch, 512-query chunk):
           S^T[m,n] = kT.T @ qT (row-tiled K=64 matmul pairs),
           P = exp(S*scale) (no max subtraction needed: scores ~ N(0,1)),
           O^T_ext[65,n] = [V|1].T @ P^T accumulated over key tiles in PSUM
           (row 64 = softmax denominator). O and den are copied to SBUF so
           the PSUM bank frees immediately; normalization (one batched
           reciprocal over all 16 denominators + broadcast + multiply)
           happens at the end of the phase.
  AllToAll: each core sends its 2-head output columns for core j's token
           rows; receives full 1024 c_in x its 512 token rows (2MB/core).
  Phase C: out rows = attnT.T @ w_proj.T + b_proj for this core's 512 rows.
Host gathers: concat rows -> [4096, 1024] -> [2, 2048, 1024].
"""
import sys, os, types
import numpy as np

if "/opt/trn_rl_repo" not in sys.path and os.path.isdir("/opt/trn_rl_repo"):
    sys.path.append("/opt/trn_rl_repo")

import concourse.bass as bass
import concourse.mybir as mybir
import concourse.tile as tile
from concourse import bacc
from concourse.bass_utils import run_bass_kernel_spmd

F32 = mybir.dt.float32
F32R = mybir.dt.float32r
BF16 = mybir.dt.bfloat16
EXP = mybir.ActivationFunctionType.Exp

NCORES = 8
B, N, C, H, D = 2, 2048, 1024, 16, 64
NT = B * N          # 4096 flat tokens
KT = C // 128       # 8 contraction tiles of 128
QC = 512            # query-chunk width (one PSUM bank)
NU = NT // QC       # 8 (batch, qchunk) units == A2A shard count
NMT = N // 128      # 16 key tiles per batch
SCALE = 1.0 / 8.0   # 1/sqrt(D)
GRP = 2             # score banks per exp call (pipeline depth 3 over 6 banks)
XCH = 1024          # x load chunk width (4KB rows for efficient DMA)

TRACE = False       # test harness sets True to capture exec_time_ns
LAST_EXEC_NS = None

_NC = None


def _round_f32r(a: np.ndarray) -> np.ndarray:
    """Round-to-nearest-even to the fp32r (e8m10) grid, matching the PE."""
    u = np.ascontiguousarray(a, dtype=np.float32).view(np.uint32)
    lsb = (u >> np.uint32(13)) & np.uint32(1)
    r = (u + np.uint32(0x0FFF) + lsb) & np.uint32(0xFFFFE000)
    return r.view(np.float32)


def _install_ntff_hook():
    if "antenv.axon_hooks" in sys.modules:
        return
    try:
        import antenv
        from trn_agent_boot.trn_boot import _ntff_profile_via_ctypes
        mod = types.ModuleType("antenv.axon_hooks")
        _hook = [None]
        mod.set_axon_ntff_profile_hook = lambda h: _hook.__setitem__(0, h)
        mod.get_axon_ntff_profile_hook = lambda: _hook[0]
        sys.modules["antenv.axon_hooks"] = mod
        antenv.axon_hooks = mod
        mod.set_axon_ntff_profile_hook(
            _ntff_profile_via_ctypes("/opt/axon/libaxon_pjrt.so"))
    except Exception:
        pass


def _build():
    nc = bacc.Bacc("TRN2", target_bir_lowering=False, debug=False,
                   num_devices=NCORES)
    xT_ext = nc.dram_tensor("xT", [C, NT], BF16, kind="ExternalInput").ap()
    wT_ext = nc.dram_tensor("wT", [C, 384], BF16, kind="ExternalInput").ap()
    wpT_ext = nc.dram_tensor("wpT", [C, C], BF16, kind="ExternalInput").ap()
    bias_ext = nc.dram_tensor("bias", [1, C], F32, kind="ExternalInput").ap()
    idn_ext = nc.dram_tensor("idn", [128, 128], BF16, kind="ExternalInput").ap()
    out_ext = nc.dram_tensor("out", [NT // NCORES, C], F32,
                             kind="ExternalOutput").ap()
    a2a_in = nc.dram_tensor("a2a_in", [NCORES * 128, QC], BF16)
    a2a_out = nc.dram_tensor("a2a_out", [NCORES * 128, QC], BF16)

    xT_v = xT_ext.rearrange("(kt p) n -> p kt n", p=128)
    wT_v = wT_ext.rearrange("(kt p) f -> p kt f", p=128)
    wpT_v = wpT_ext.rearrange("(kt p) f -> p kt f", p=128)

    with tile.TileContext(nc) as tc:
        with (
            tc.tile_pool(name="const", bufs=1) as cpool,
            tc.tile_pool(name="resid", bufs=1) as rpool,
        ):
            wT_sb = cpool.tile([128, KT, 384], BF16)
            for kt in range(KT):
                nc.sync.dma_start(wT_sb[:, kt, :], wT_v[:, kt, :])
            idn = cpool.tile([128, 128], BF16)
            nc.sync.dma_start(idn[:], idn_ext[:])
            bias_sb = cpool.tile([1, C], F32)
            nc.sync.dma_start(bias_sb[:], bias_ext[:])
            bias_bc = cpool.tile([128, C], F32)
            nc.gpsimd.partition_broadcast(bias_bc[:], bias_sb[:])

            qT_sb = rpool.tile([128, NT], BF16)
            kT_sb = rpool.tile([128, NT], BF16)
            v_sb = rpool.tile([128, NT // 128, 130], BF16)
            nc.gpsimd.memset(v_sb[:, :, 64], 1.0)
            nc.gpsimd.memset(v_sb[:, :, 129], 1.0)
            # unnormalized attention outputs: block (u, h) lives at
            # [0:64, u*2+h, :] (base partition 0 so DVE ops can pair it
            # with broadcast tiles)
            stage = rpool.tile([64, 2 * NU, QC], F32)
            wp_sb = rpool.tile([128, KT, C], BF16)

            def qkv_groups(vpool, apsum, tpsum, x_tiles, bat, psum_tag):
                """Yield one closure per QKV matmul-group (8 accumulating
                matmuls + PSUM evacuation; the v-feature groups also emit
                the PE transposes building V-natural)."""
                for nch2 in range(N // XCH):
                    x_t = x_tiles[bat * (N // XCH) + nch2]
                    for hw in range(XCH // QC):
                        ncol = bat * N + nch2 * XCH + hw * QC
                        for ft in range(3):
                            def emit(ncol=ncol, hw=hw, ft=ft, x_t=x_t):
                                xs = x_t[:, :, hw * QC:(hw + 1) * QC]
                                ps = apsum.tile([128, QC], F32, tag=psum_tag,
                                                name=f"qkv_{ncol}_{ft}")
                                for kt in range(KT):
                                    nc.tensor.matmul(
                                        ps[:],
                                        wT_sb[:, kt, ft * 128:(ft + 1) * 128],
                                        xs[:, kt, :],
                                        start=(kt == 0), stop=(kt == KT - 1))
                                if ft == 0:
                                    nc.vector.tensor_copy(
                                        out=qT_sb[:, ncol:ncol + QC],
                                        in_=ps[:])
                                elif ft == 1:
                                    nc.vector.tensor_copy(
                                        out=kT_sb[:, ncol:ncol + QC],
                                        in_=ps[:])
                                else:
                                    vt = vpool.tile([128, QC], BF16, tag="vt",
                                                    name=f"vt_{ncol}")
                                    nc.vector.tensor_copy(out=vt[:],
                                                          in_=ps[:])
                                    for t in range(4):
                                        mtg = ncol // 128 + t
                                        trp = tpsum.tile(
                                            [128, 128], BF16, tag="tr",
                                            name=f"tr_{mtg}")
                                        nc.tensor.transpose(
                                            trp[:],
                                            vt[:, t * 128:(t + 1) * 128],
                                            idn[:])
                                        nc.vector.tensor_copy(
                                            out=v_sb[:, mtg, 0:64],
                                            in_=trp[:, 0:64])
                                        nc.vector.tensor_copy(
                                            out=v_sb[:, mtg, 65:129],
                                            in_=trp[:, 64:128])
                            yield emit

            def attn_phase(spsum, opsum, ppool, denpool, rbpool, onpool,
                           bat, fillers=(), fill_every=3):
                fillers = list(fillers)
                fill_count = 0
                for uu in range(N // QC):
                    u = bat * (N // QC) + uu
                    qcol = u * QC
                    # interleave heads so row-tiled K=64 QK^T pairs
                    # (PE tiles T0/T8) run concurrently
                    units = [(h, mt) for mt in range(NMT) for h in range(2)]
                    o_cur = {}
                    for g0 in range(0, len(units), GRP):
                        g = units[g0:g0 + GRP]
                        s_t = spsum.tile([128, GRP, QC], F32, tag="s",
                                         name=f"s_{u}_{g0}")
                        for ui, (h, mt) in enumerate(g):
                            if mt == 0 and h not in o_cur:
                                o_cur[h] = opsum.tile(
                                    [65, QC], F32, tag=f"o{h}",
                                    name=f"o_ps{h}_{u}")
                            nc.tensor.matmul(
                                s_t[:, ui, :],
                                kT_sb[h * 64:(h + 1) * 64,
                                      bat * N + mt * 128:
                                      bat * N + (mt + 1) * 128],
                                qT_sb[h * 64:(h + 1) * 64, qcol:qcol + QC],
                                start=True, stop=True)
                        p_t = ppool.tile([128, GRP, QC], BF16, tag="p",
                                         name=f"p_{u}_{g0}")
                        nc.scalar.activation(p_t[:, 0:len(g), :],
                                             s_t[:, 0:len(g), :],
                                             EXP, scale=SCALE)
                        for ui, (h, mt) in enumerate(g):
                            nc.tensor.matmul(
                                o_cur[h][:],
                                v_sb[:, bat * NMT + mt, h * 65:(h + 1) * 65],
                                p_t[:, ui, :],
                                start=(mt == 0), stop=(mt == NMT - 1))
                            if mt == NMT - 1:
                                o_ps = o_cur.pop(h)
                                nc.vector.tensor_copy(
                                    out=stage[:, u * 2 + h, :],
                                    in_=o_ps[0:64, :])
                                den = denpool.tile([1, QC], F32, tag="den",
                                                   name=f"den_{u}_{h}")
                                nc.vector.tensor_copy(out=den[:],
                                                      in_=o_ps[64:65, :])
                                rcp = denpool.tile([1, QC], F32, tag="rcp",
                                                   name=f"rcp_{u}_{h}")
                                nc.vector.reciprocal(rcp[:], den[:])
                                rb = rbpool.tile([64, QC], F32, tag="rb",
                                                 name=f"rb_{u}_{h}")
                                nc.gpsimd.partition_broadcast(rb[:], rcp[:])
                                o_n = onpool.tile([64, QC], BF16, tag="on",
                                                  name=f"on_{u}_{h}")
                                nc.vector.tensor_tensor(
                                    o_n[:], stage[:, u * 2 + h, :],
                                    rb[:], mybir.AluOpType.mult)
                                nc.sync.dma_start(
                                    a2a_in[u * 128 + h * 64:
                                           u * 128 + (h + 1) * 64, :],
                                    o_n[:])
                        fill_count += 1
                        if fillers and fill_count % fill_every == 0:
                            fillers.pop(0)()
                for f in fillers:
                    f()

            with (
                tc.tile_pool(name="xchunk", bufs=1) as xpool,
                tc.tile_pool(name="vtmp", bufs=2) as vpool,
                tc.tile_pool(name="pexp", bufs=4) as ppool,
                tc.tile_pool(name="denp", bufs=4) as denpool,
                tc.tile_pool(name="rbp", bufs=4) as rbpool,
                tc.tile_pool(name="onrm", bufs=4) as onpool,
            ):
                # batch-0 x chunks load first; batch-1 chunk DMAs overlap
                # batch-0 attention (no PSUM involved in a DMA)
                x_tiles = []
                for nch in range(NT // XCH):
                    x_t = xpool.tile([128, KT, XCH], BF16, tag=f"x{nch}",
                                     name=f"x_{nch}")
                    x_tiles.append(x_t)
                for nch in range(NT // XCH):
                    for kt in range(KT):
                        nc.sync.dma_start(
                            x_tiles[nch][:, kt, :],
                            xT_v[:, kt, nch * XCH:(nch + 1) * XCH])

                for bat in range(B):
                    with (
                        tc.tile_pool(name=f"qkvps{bat}", bufs=2,
                                     space="PSUM") as apsum,
                        tc.tile_pool(name=f"trps{bat}", bufs=2,
                                     space="PSUM") as tpsum,
                    ):
                        for emit in qkv_groups(vpool, apsum, tpsum, x_tiles,
                                               bat, f"a{bat}"):
                            emit()
                    if bat == 0:
                        # w_proj load rides the attention-phase DMA idle
                        nc.sync.dma_start(wp_sb[:], wpT_v[:])
                    with (
                        tc.tile_pool(name=f"sps{bat}", bufs=3,
                                     space="PSUM") as spsum,
                        tc.tile_pool(name=f"ops{bat}", bufs=1,
                                     space="PSUM") as opsum,
                    ):
                        attn_phase(spsum, opsum, ppool, denpool, rbpool,
                                   onpool, bat)

            nc.gpsimd.collective_compute(
                "AllToAll",
                mybir.AluOpType.bypass,
                replica_groups=[list(range(NCORES))],
                ins=[a2a_in[:]],
                outs=[a2a_out[:]],
            )

            # ---- Phase C: output projection for this core's 512 rows ----
            with (
                tc.tile_pool(name="plhs", bufs=1) as lpool,
                tc.tile_pool(name="pps", bufs=1, space="PSUM") as ppsum,
                tc.tile_pool(name="pout", bufs=2) as outpool,
            ):
                lhs = lpool.tile([128, KT, QC], BF16)
                for j in range(KT):
                    nc.sync.dma_start(lhs[:, j, :],
                                      a2a_out[j * 128:(j + 1) * 128, :])
                pp = {i: ppsum.tile([128, QC], F32, tag=f"pp{i}",
                                    name=f"pp_{i}")
                      for i in range(8)}
                for j in range(KT):
                    for mt in range(4):
                        for half in range(2):
                            nc.tensor.matmul(
                                pp[mt * 2 + half][:],
                                lhs[:, j, mt * 128:(mt + 1) * 128],
                                wp_sb[:, j, half * QC:(half + 1) * QC],
                                start=(j == 0), stop=(j == KT - 1))
                for mt in range(4):
                    for half in range(2):
                        ot = outpool.tile([128, QC], F32, tag="ot",
                                          name=f"ot_{mt}_{half}")
                        nc.vector.tensor_tensor(
                            ot[:], pp[mt * 2 + half][:],
                            bias_bc[:, half * QC:(half + 1) * QC],
                            mybir.AluOpType.add)
                        nc.sync.dma_start(
                            out_ext[mt * 128:(mt + 1) * 128,
                                    half * QC:(half + 1) * QC],
                            ot[:])
    nc.compile()
    return nc


def kernel(x, w_qkv, w_proj, b_proj):
    global _NC, LAST_EXEC_NS
    if _NC is None:
        _NC = _build()
    x = np.asarray(x, dtype=np.float32)
    w_qkv = np.asarray(w_qkv, dtype=np.float32)
    w_proj = np.asarray(w_proj, dtype=np.float32)
    b_proj = np.asarray(b_proj, dtype=np.float32)

    import ml_dtypes
    xT = np.ascontiguousarray(x.reshape(NT, C).T).astype(ml_dtypes.bfloat16)
    wpT = np.ascontiguousarray(w_proj.T).astype(ml_dtypes.bfloat16)
    bias = np.ascontiguousarray(b_proj.reshape(1, C))
    idn = np.eye(128, dtype=ml_dtypes.bfloat16)
    in_maps = []
    for c in range(NCORES):
        blk = slice(128 * c, 128 * (c + 1))
        wT = np.ascontiguousarray(
            np.concatenate([w_qkv[0:C][blk], w_qkv[C:2 * C][blk],
                            w_qkv[2 * C:3 * C][blk]], axis=0).T).astype(
                ml_dtypes.bfloat16)
        in_maps.append({"xT": xT, "wT": wT, "wpT": wpT, "bias": bias,
                        "idn": idn})

    if TRACE:
        _install_ntff_hook()
    res = run_bass_kernel_spmd(_NC, in_maps, core_ids=list(range(NCORES)),
                               trace=TRACE)
    LAST_EXEC_NS = res.exec_time_ns
    out = np.concatenate([res.results[i]["out"] for i in range(NCORES)],
                         axis=0)
    return np.ascontiguousarray(out.reshape(B, N, C).astype(np.float32))



# revision 15
# speedup vs baseline: 1.0382x; 1.0382x over previous
"""Distributed multi-head attention for 8 TRN2 NeuronCores.

Problem: x[2,2048,1024] -> QKV proj (w_qkv[3072,1024]) -> 16-head SDPA ->
out proj (w_proj[1024,1024] + b_proj) -> [2,2048,1024].

Sharding: 2 heads per core (head-parallel over all 8 cores; both batches on
every core); output token rows split so core j owns tokens
[b*2048 + half*1024 + j*128, +128) for every (batch, half) -- i.e. 128
tokens per half-batch, 512 rows total.

Per core schedule (single persistent TileContext, pools never close):
  QKV(b): 12 groups of 8 accumulating matmuls -> qT/kT [128, 2048] per
          batch; V-natural via PE transpose with a ones row appended
          ([V|1], 65 cols per head) so P@[V|1] yields the softmax
          denominator for free in row 64.
  ATTN(b): per (qchunk, key-tile): S^T = kT.T @ qT (two K=64 row-tiled
          matmuls), P = exp(S/8) on the scalar engine, O^T[65,512]
          accumulated in PSUM. At the last key tile the UNNORMALIZED
          [65,512] block (row 64 = denominator) is cast to bf16 and
          DMA'd into the per-half-batch AllToAll staging buffer.
  4 AllToAlls (one per (batch, half)), each triggered as soon as its
          1024 tokens are staged, so all but the last overlap compute.
  PhaseC(hb): after A2A hb lands: batched reciprocal of the 16
          denominators [16,128], gpsimd partition-broadcast + DVE
          multiply to normalize, then out = attnT.T @ w_proj.T + bias
          for this core's 128 tokens. PhaseC work and QKV(b=1) are
          interleaved as fillers inside the scalar-bound attention
          loops to keep the PE continuously busy (p-state).
Host gathers: per core 4 sections of 128 token rows -> [2,2048,1024].
"""
import sys, os, types
import numpy as np

if "/opt/trn_rl_repo" not in sys.path and os.path.isdir("/opt/trn_rl_repo"):
    sys.path.append("/opt/trn_rl_repo")

import concourse.bass as bass
import concourse.mybir as mybir
import concourse.tile as tile
from concourse import bacc
from concourse.bass_utils import run_bass_kernel_spmd

F32 = mybir.dt.float32
BF16 = mybir.dt.bfloat16
EXP = mybir.ActivationFunctionType.Exp

NCORES = 8
B, N, C, H, D = 2, 2048, 1024, 16, 64
NT = B * N          # 4096 flat tokens
KT = C // 128       # 8 contraction tiles of 128
QC = 512            # query-chunk width
NMT = N // 128      # 16 key tiles per batch
SCALE = 1.0 / 8.0   # 1/sqrt(D)
XCH = 512           # x load chunk width
NXC = N // XCH      # 4 chunks per batch
TOK = 128           # tokens owned per core per half-batch
NHB = 4             # half-batches (a2a units)

TRACE = False       # test harness sets True to capture exec_time_ns
LAST_EXEC_NS = None

_NC = None


def _install_ntff_hook():
    if "antenv.axon_hooks" in sys.modules:
        return
    try:
        import antenv
        from trn_agent_boot.trn_boot import _ntff_profile_via_ctypes
        mod = types.ModuleType("antenv.axon_hooks")
        _hook = [None]
        mod.set_axon_ntff_profile_hook = lambda h: _hook.__setitem__(0, h)
        mod.get_axon_ntff_profile_hook = lambda: _hook[0]
        sys.modules["antenv.axon_hooks"] = mod
        antenv.axon_hooks = mod
        mod.set_axon_ntff_profile_hook(
            _ntff_profile_via_ctypes("/opt/axon/libaxon_pjrt.so"))
    except Exception:
        pass


def _build():
    nc = bacc.Bacc("TRN2", target_bir_lowering=False, debug=False,
                   num_devices=NCORES)
    xT_ext = nc.dram_tensor("xT", [C, NT], BF16, kind="ExternalInput").ap()
    wT_ext = nc.dram_tensor("wT", [C, 384], BF16, kind="ExternalInput").ap()
    wpT_ext = nc.dram_tensor("wpT", [C, C], BF16, kind="ExternalInput").ap()
    bias_ext = nc.dram_tensor("bias", [1, C], F32, kind="ExternalInput").ap()
    idn_ext = nc.dram_tensor("idn", [128, 128], BF16, kind="ExternalInput").ap()
    sel_ext = nc.dram_tensor("sel", [16, KT * 128], F32,
                             kind="ExternalInput").ap()
    out_ext = nc.dram_tensor("out", [NHB * TOK, C], BF16,
                             kind="ExternalOutput").ap()
    a2a_in = [nc.dram_tensor(f"a2a_in{i}", [NCORES * 130, TOK], BF16)
              for i in range(NHB)]
    a2a_out = [nc.dram_tensor(f"a2a_out{i}", [NCORES * 130, TOK], BF16)
               for i in range(NHB)]

    xT_v = xT_ext.rearrange("(kt p) n -> p kt n", p=128)
    wT_v = wT_ext.rearrange("(kt p) f -> p kt f", p=128)
    wpT_v = wpT_ext.rearrange("(kt p) f -> p kt f", p=128)

    with tile.TileContext(nc) as tc:
        with (
            tc.tile_pool(name="const", bufs=1) as cpool,
            tc.tile_pool(name="resid", bufs=1) as rpool,
            tc.tile_pool(name="xchunk", bufs=1) as xpool,
            tc.tile_pool(name="vtmp", bufs=2) as vpool,
            tc.tile_pool(name="pexp", bufs=4) as ppool,
            tc.tile_pool(name="ostg", bufs=4) as stpool,
            tc.tile_pool(name="cden", bufs=2) as dpool,
            tc.tile_pool(name="clhs", bufs=2) as lpool,

            tc.tile_pool(name="cout", bufs=4) as outpool,
            tc.tile_pool(name="spsum", bufs=2, space="PSUM") as spool,
            tc.tile_pool(name="opsum", bufs=1, space="PSUM") as opool,
            tc.tile_pool(name="mpsum", bufs=2, space="PSUM") as mpool,
        ):
            # ---- constants ----
            wT_sb = cpool.tile([128, KT, 384], BF16)
            for kt in range(KT):
                nc.sync.dma_start(wT_sb[:, kt, :], wT_v[:, kt, :])
            idn = cpool.tile([128, 128], BF16)
            nc.sync.dma_start(idn[:], idn_ext[:])
            bias_sb = cpool.tile([1, C], F32)
            nc.sync.dma_start(bias_sb[:], bias_ext[:])
            bias_bc = cpool.tile([128, C], F32)
            nc.gpsimd.partition_broadcast(bias_bc[:], bias_sb[:])
            sel_sb = cpool.tile([16, KT, 128], F32)
            nc.sync.dma_start(sel_sb[:], sel_ext[:])

            qT_sb = [rpool.tile([128, N], BF16, name=f"qT{b}")
                     for b in range(B)]
            kT_sb = [rpool.tile([128, N], BF16, name=f"kT{b}")
                     for b in range(B)]
            v_sb = [rpool.tile([128, NMT, 130], BF16, name=f"v{b}")
                    for b in range(B)]
            for b in range(B):
                nc.gpsimd.memset(v_sb[b][:, :, 64], 1.0)
                nc.gpsimd.memset(v_sb[b][:, :, 129], 1.0)
            wp_sb = rpool.tile([128, KT, C], BF16)

            # ---- x loads: batch 0 chunks first, then w_proj, then b1 ----
            x_tiles = {}
            for b in range(B):
                for nch in range(NXC):
                    x_tiles[(b, nch)] = xpool.tile(
                        [128, KT, XCH], BF16, tag=f"x{b}{nch}",
                        name=f"x_{b}_{nch}")
            for nch in range(NXC):
                for kt in range(KT):
                    nc.sync.dma_start(
                        x_tiles[(0, nch)][:, kt, :],
                        xT_v[:, kt, nch * XCH:(nch + 1) * XCH])
            nc.sync.dma_start(wp_sb[:], wpT_v[:])
            for nch in range(NXC):
                for kt in range(KT):
                    nc.sync.dma_start(
                        x_tiles[(1, nch)][:, kt, :],
                        xT_v[:, kt, N + nch * XCH:N + (nch + 1) * XCH])

            def qkv_group(b, nch, ft):
                """One QKV matmul group: 8 accumulating matmuls + PSUM
                evacuation (v-feature groups also emit the PE transposes
                building V-natural)."""
                ncol = nch * XCH
                xs = x_tiles[(b, nch)]
                ps = mpool.tile([128, QC], F32, tag="mm",
                                name=f"qkv_{b}_{ncol}_{ft}")
                for kt in range(KT):
                    nc.tensor.matmul(
                        ps[:],
                        wT_sb[:, kt, ft * 128:(ft + 1) * 128],
                        xs[:, kt, :],
                        start=(kt == 0), stop=(kt == KT - 1))
                if ft == 0:
                    nc.vector.tensor_copy(out=qT_sb[b][:, ncol:ncol + QC],
                                          in_=ps[:])
                elif ft == 1:
                    nc.vector.tensor_copy(out=kT_sb[b][:, ncol:ncol + QC],
                                          in_=ps[:])
                else:
                    vt = vpool.tile([128, QC], BF16, tag="vt",
                                    name=f"vt_{b}_{ncol}")
                    nc.vector.tensor_copy(out=vt[:], in_=ps[:])
                    for t in range(4):
                        mtg = ncol // 128 + t
                        trp = mpool.tile([128, 128], BF16, tag="mm",
                                         name=f"tr_{b}_{mtg}")
                        nc.tensor.transpose(
                            trp[:], vt[:, t * 128:(t + 1) * 128], idn[:])
                        nc.vector.tensor_copy(out=v_sb[b][:, mtg, 0:64],
                                              in_=trp[:, 0:64])
                        nc.vector.tensor_copy(out=v_sb[b][:, mtg, 65:129],
                                              in_=trp[:, 64:128])

            def phase_c(hb):
                """Output projection for this core's 128 tokens of
                half-batch hb, split into filler closures. Returns the
                closure list; caller schedules them after A2A hb lands."""
                ao = a2a_out[hb].ap()
                den_v = ao.rearrange("(j r) t -> j r t", r=130)

                den = dpool.tile([16, TOK], BF16, tag="den",
                                 name=f"den_{hb}")
                denf = dpool.tile([16, TOK], F32, tag="denf",
                                  name=f"denf_{hb}")
                rcp = dpool.tile([16, TOK], F32, tag="rcp",
                                 name=f"rcp_{hb}")
                lhs = lpool.tile([128, KT, TOK], BF16, tag="lhs",
                                 name=f"lhs_{hb}")
                lhs_n = lpool.tile([128, KT, TOK], BF16, tag="lhsn",
                                   name=f"lhsn_{hb}")

                def c_load():
                    # denominators: rows j*130 + h*65 + 64
                    nc.gpsimd.dma_start(den[0:8, :], den_v[:, 64, :])
                    nc.gpsimd.dma_start(den[8:16, :], den_v[:, 129, :])
                    nc.vector.tensor_copy(out=denf[:], in_=den[:])
                    nc.vector.reciprocal(rcp[:], denf[:])
                    for kt in range(KT):
                        nc.gpsimd.dma_start(
                            lhs[0:64, kt, :],
                            ao[kt * 130:kt * 130 + 64, :])
                        nc.gpsimd.dma_start(
                            lhs[64:128, kt, :],
                            ao[kt * 130 + 65:kt * 130 + 129, :])

                def c_norm(k0, k1):
                    def f():
                        for kt in range(k0, k1):
                            # broadcast rcp rows (kt, 8+kt) to a
                            # [128, TOK] tile via a tiny selector matmul
                            # (engine APs can't start at odd partitions)
                            rb = spool.tile([128, TOK], F32, tag="s",
                                            name=f"rb_{hb}_{kt}")
                            nc.tensor.matmul(
                                rb[:], sel_sb[:, kt, :], rcp[:],
                                start=True, stop=True)
                            nc.vector.tensor_tensor(
                                lhs_n[:, kt, :], lhs[:, kt, :], rb[:],
                                mybir.AluOpType.mult)
                    return f

                pp = {}

                def c_mm(k0, k1):
                    def f():
                        for half in range(2):
                            if k0 == 0:
                                pp[half] = mpool.tile(
                                    [128, QC], F32, tag="mm",
                                    name=f"pp_{hb}_{half}")
                            for kt in range(k0, k1):
                                nc.tensor.matmul(
                                    pp[half][:],
                                    lhs_n[:, kt, :],
                                    wp_sb[:, kt, half * QC:(half + 1) * QC],
                                    start=(kt == 0), stop=(kt == KT - 1))
                    return f

                def c_out():
                    for half in range(2):
                        ot = outpool.tile([TOK, QC], BF16, tag="ot",
                                          name=f"ot_{hb}_{half}")
                        nc.vector.tensor_tensor(
                            ot[:], pp[half][:],
                            bias_bc[0:TOK, half * QC:(half + 1) * QC],
                            mybir.AluOpType.add)
                        nc.gpsimd.dma_start(
                            out_ext[hb * TOK:(hb + 1) * TOK,
                                    half * QC:(half + 1) * QC],
                            ot[:])

                return [c_load, c_norm(0, 4), c_mm(0, 4), c_norm(4, 8),
                        c_mm(4, 8), c_out]

            def attn_phase(b, fillers):
                """Attention for batch b. fillers: dict step->list of
                closures, inserted after that step's PV emission.
                Steps are numbered 0..63 (qchunk*16 + mt)."""
                pend = []  # software-pipelined PV emission

                def flush_pv():
                    for f in pend:
                        f()
                    pend.clear()

                o_cur = {}
                for q in range(NXC):
                    qcol = q * QC
                    for mt in range(NMT):
                        step = q * NMT + mt
                        s_t = spool.tile([128, 2, QC], F32, tag="s",
                                         name=f"s_{b}_{step}")
                        for h in range(2):
                            nc.tensor.matmul(
                                s_t[:, h, :],
                                kT_sb[b][h * 64:(h + 1) * 64,
                                         mt * 128:(mt + 1) * 128],
                                qT_sb[b][h * 64:(h + 1) * 64,
                                         qcol:qcol + QC],
                                start=True, stop=True)
                        flush_pv()
                        p_t = ppool.tile([128, 2, QC], BF16, tag="p",
                                         name=f"p_{b}_{step}")
                        nc.scalar.activation(p_t[:], s_t[:], EXP,
                                             scale=SCALE)

                        def pv(mt=mt, q=q, p_t=p_t):
                            for h in range(2):
                                if mt == 0:
                                    o_cur[h] = opool.tile(
                                        [65, QC], F32, tag=f"o{h}",
                                        name=f"o_{b}_{q}_{h}")
                                nc.tensor.matmul(
                                    o_cur[h][:],
                                    v_sb[b][:, mt, h * 65:(h + 1) * 65],
                                    p_t[:, h, :],
                                    start=(mt == 0), stop=(mt == NMT - 1))
                                if mt == NMT - 1:
                                    o_ps = o_cur.pop(h)
                                    stg = stpool.tile(
                                        [65, QC], BF16, tag="st",
                                        name=f"st_{b}_{q}_{h}")
                                    nc.vector.tensor_copy(out=stg[:],
                                                          in_=o_ps[:])
                                    hb = b * 2 + q // 2
                                    for dd in range(4):
                                        j = (q % 2) * 4 + dd
                                        nc.sync.dma_start(
                                            a2a_in[hb][
                                                j * 130 + h * 65:
                                                j * 130 + (h + 1) * 65, :],
                                            stg[:, dd * TOK:(dd + 1) * TOK])
                        pend.append(pv)
                        for f in fillers.get(step, ()):  # noqa: B023
                            f()
                    if q % 2 == 1:
                        hb = b * 2 + q // 2
                        flush_pv()
                        nc.gpsimd.collective_compute(
                            "AllToAll",
                            mybir.AluOpType.bypass,
                            replica_groups=[list(range(NCORES))],
                            ins=[a2a_in[hb].ap()],
                            outs=[a2a_out[hb].ap()],
                        )
                flush_pv()
                # run any fillers scheduled past the last step
                for step in sorted(k for k in fillers if k >= NXC * NMT):
                    for f in fillers[step]:
                        f()

            # ---- schedule ----
            for nch in range(NXC):
                for ft in range(3):
                    qkv_group(0, nch, ft)

            # attn b0: interleave QKV b1 groups every 4 steps
            fill0 = {}
            i = 0
            for nch in range(NXC):
                for ft in range(3):
                    fill0.setdefault(4 + 4 * i, []).append(
                        lambda nch=nch, ft=ft: qkv_group(1, nch, ft))
                    i += 1
            attn_phase(0, fill0)

            # attn b1: interleave phase C for hb0 (landed during b0),
            # hb1 (triggered at b0 end), hb2 (triggered mid-b1)
            fill1 = {}
            for ci, cl in enumerate(phase_c(0)):
                fill1.setdefault(2 + 2 * ci, []).append(cl)
            for ci, cl in enumerate(phase_c(1)):
                fill1.setdefault(26 + 2 * ci, []).append(cl)
            for ci, cl in enumerate(phase_c(2)):
                fill1.setdefault(52 + 2 * ci, []).append(cl)
            attn_phase(1, fill1)

            # tail: last half-batch
            for cl in phase_c(3):
                cl()
    nc.compile()
    return nc


def kernel(x, w_qkv, w_proj, b_proj):
    global _NC, LAST_EXEC_NS
    if _NC is None:
        _NC = _build()
    x = np.asarray(x, dtype=np.float32)
    w_qkv = np.asarray(w_qkv, dtype=np.float32)
    w_proj = np.asarray(w_proj, dtype=np.float32)
    b_proj = np.asarray(b_proj, dtype=np.float32)

    import ml_dtypes
    xT = np.ascontiguousarray(x.reshape(NT, C).T).astype(ml_dtypes.bfloat16)
    wpT = np.ascontiguousarray(w_proj.T).astype(ml_dtypes.bfloat16)
    bias = np.ascontiguousarray(b_proj.reshape(1, C))
    idn = np.eye(128, dtype=ml_dtypes.bfloat16)
    # rcp partition layout: rows 0..7 = h0 dens (head 2j), 8..15 = h1
    # dens (head 2j+1); channel block kt holds heads (2kt, 2kt+1)
    sel = np.zeros((16, KT * 128), dtype=np.float32)
    for kt in range(KT):
        sel[kt, kt * 128:kt * 128 + 64] = 1.0
        sel[8 + kt, kt * 128 + 64:kt * 128 + 128] = 1.0
    in_maps = []
    for c in range(NCORES):
        blk = slice(128 * c, 128 * (c + 1))
        wT = np.ascontiguousarray(
            np.concatenate([w_qkv[0:C][blk], w_qkv[C:2 * C][blk],
                            w_qkv[2 * C:3 * C][blk]], axis=0).T).astype(
                ml_dtypes.bfloat16)
        in_maps.append({"xT": xT, "wT": wT, "wpT": wpT, "bias": bias,
                        "idn": idn, "sel": sel})

    if TRACE:
        _install_ntff_hook()
    res = run_bass_kernel_spmd(_NC, in_maps, core_ids=list(range(NCORES)),
                               trace=TRACE)
    LAST_EXEC_NS = res.exec_time_ns
    out = np.empty((B, N, C), dtype=np.float32)
    for j in range(NCORES):
        o = np.asarray(res.results[j]["out"]).astype(np.float32)
        for hb in range(NHB):
            b, half = hb // 2, hb % 2
            t0 = half * 1024 + j * TOK
            out[b, t0:t0 + TOK, :] = o[hb * TOK:(hb + 1) * TOK, :]
    return np.ascontiguousarray(out)
